# revision 7
# baseline (speedup 1.0000x reference)
"""Haar DWT on 8 Trainium2 NeuronCores (batch-parallel, 1 image per core).

Layout: partition p of tile t holds 16 consecutive input rows (8 output
rows) of one channel: 16-row block g2 = 128*t + p, channel c = g2//32,
rows 16*(g2%32)..+16. Tile free dim = 8192 (16 rows x 512 cols), 4 MiB.

Fewer, bigger DMAs than the 2 MiB/tile variant: per-dma completion
semaphore descriptors cost ~500 ns (loads) / ~110 ns (stores) on every
SDMA engine, so halving the dma_start count reclaims ~15 us of
per-engine DMA busy time. Store descriptors double to 8 KiB (8
consecutive output rows per partition per subband).

Pools: pin bufs=2 (8 MiB), psd bufs=2 (4 MiB), pout bufs=3 (12 MiB).
The 3-deep pout ring is the key: with only 2 out-buffers, stage 2 of
tile t WAR-waits on the stores of tile t-2, costing ~5% of DVE
occupancy (88% -> 93% measured when deepened). pin=2 trades a little
load lookahead for that SBUF, a measured net win (343 vs 353 us).

Per-core pipeline, 16 tiles (4 channels each), software-pipelined
(loads issued 2 tiles ahead, the ScalarE 0.5 mul 1 tile ahead, so the
ACT queue never head-of-line-blocks a ready mul behind a store that is
still waiting on DVE):
  1. in-DMA: 4 MiB fully contiguous, 32 KiB per-partition descriptors
     (SP HWDGE ring)
  2. ScalarE in-place x *= 0.5 (exact in fp32; folds the Haar scale)
  3. compute in two 4096-elem halves (8 rows each) to keep the sd pool
     small; per half:
       DVE stage 1 (column butterfly, stride-2 views, FD=2048):
         sum1 = x[0::2] + x[1::2], diff1 = x[1::2] - x[0::2]
       DVE stage 2 (row butterfly, FD=2048):
         add -> LL + HL, sub -> LH + HH
     (all-DVE: concurrent GpSimd tensor ops contend with DVE on the
     shared SBUF port pair — measured 2.2 -> 6.5 us per op — and
     tensor_tensor_reduce, which would fold the 0.5 scale into stage 1,
     fails at runtime on HW, so ScalarE keeps the scale pass)
     o_sb layout [sb:4][j:8][w:256]: per partition each subband block
     is 8 KiB = 8 consecutive output rows, contiguous in DRAM
  4. two 2 MiB out-DMAs per tile ({LL,HL} / {LH,HH}; 3-dim APs, 8 KiB
     per-partition descriptors) on the ACT HWDGE ring

Engine busy per core (clean-window trace): DVE ~284 us at 93%
occupancy, ScalarE ~115 us, DMA ~318 us/engine at descriptor line
rate. Measured 343-350 us; the residue over the DMA floor is ~8 us of
startup ramp and the DVE-gated final stores.
"""

import sys

sys.path.insert(0, "/opt/trn_rl_repo")

import numpy as np

import concourse.bass as bass
import concourse.bacc as bacc
import concourse.mybir as mybir
from concourse import tile
from concourse.bass_utils import run_bass_kernel_spmd

N_CORES = 8
C = 64
H = 512
W = 512
HO = H // 2
WO = W // 2
P = 128
FD = 8192                      # 16 input rows per partition
TILES = C * H * W // (P * FD)  # 16
OFD = FD // 4                  # 2048: out elems per partition per subband
HFD = FD // 2                  # 4096: half-tile free dim

F32 = mybir.dt.float32


def build_nc() -> bass.Bass:
    nc = bacc.Bacc()
    x = nc.dram_tensor("x", [C, H, W], F32, kind="ExternalInput")
    out = nc.dram_tensor("out", [4 * C, HO, WO], F32, kind="ExternalOutput")

    # [2048 row-blocks, 8192]: block g2 = (c, hb), free = (r:16, w:512)
    x_v = x.rearrange("c (hb r) w -> (c hb) (r w)", r=16)
    # per subband: out[sb*64 + cc, h, w] flattened — block g2 owns the
    # contiguous 2048-elem range starting at g2*2048
    out_v = out.rearrange("(s cc) h w -> s (cc h w)", s=4)

    with tile.TileContext(nc) as tc:
        with (
            tc.tile_pool(name="pin", bufs=2) as pin,
            tc.tile_pool(name="psd", bufs=2) as psd,
            tc.tile_pool(name="pout", bufs=3) as pout,
        ):
            ins: dict[int, object] = {}

            def issue_load(t):
                in_sb = pin.tile([P, FD], F32)
                ins[t] = in_sb
                nc.sync.dma_start(in_sb[:], x_v[t * P : (t + 1) * P, :])

            def issue_scale(t):
                in_sb = ins[t]
                nc.scalar.mul(in_sb[:], in_sb[:], 0.5)

            def issue_rest(t):
                in_sb = ins.pop(t)
                o_sb = pout.tile([P, FD], F32)
                o4 = o_sb[:].rearrange("p (sb j w) -> p sb j w", sb=4, j=8)
                for h in range(2):
                    sd = psd.tile([P, HFD], F32)
                    i3 = in_sb[:, h * HFD : (h + 1) * HFD].rearrange(
                        "p (k two) -> p k two", two=2
                    )
                    nc.vector.tensor_add(sd[:, 0:2048], i3[:, :, 0], i3[:, :, 1])
                    nc.vector.tensor_sub(sd[:, 2048:4096], i3[:, :, 1], i3[:, :, 0])

                    # sd: [half2][j:4][parity:2][w:256]
                    s4 = sd[:].rearrange(
                        "p (half j parity w) -> p half j parity w",
                        half=2, j=4, parity=2,
                    )
                    o4h = o4[:, :, 4 * h : 4 * h + 4, :]
                    # LL (sb0) from sum-half, HL (sb2) from diff-half
                    nc.vector.tensor_add(
                        o4h[:, 0::2, :, :], s4[:, :, :, 0, :], s4[:, :, :, 1, :]
                    )
                    # LH (sb1) from sum-half, HH (sb3) from diff-half
                    nc.vector.tensor_sub(
                        o4h[:, 1::2, :, :], s4[:, :, :, 1, :], s4[:, :, :, 0, :]
                    )

                # two 2 MiB stores per tile (ACT ring)
                dst = out_v[:, t * P * OFD : (t + 1) * P * OFD].rearrange(
                    "s (p f) -> p s f", f=OFD
                )
                src4 = o_sb[:].rearrange("p (s f) -> p s f", s=4)
                nc.scalar.dma_start(dst[:, 0::2, :], src4[:, 0::2, :])
                nc.scalar.dma_start(dst[:, 1::2, :], src4[:, 1::2, :])

            issue_load(0)
            issue_load(1)
            issue_scale(0)
            for t in range(TILES):
                if t + 2 < TILES:
                    issue_load(t + 2)
                if t + 1 < TILES:
                    issue_scale(t + 1)
                issue_rest(t)

    nc.finalize()
    return nc


_NC_CACHE: dict = {}


def _get_nc() -> bass.Bass:
    if "nc" not in _NC_CACHE:
        _NC_CACHE["nc"] = build_nc()
    return _NC_CACHE["nc"]


def kernel(x: np.ndarray) -> np.ndarray:
    x = np.asarray(x)
    assert x.shape == (N_CORES, C, H, W), x.shape
    nc = _get_nc()
    in_maps = [{"x": np.ascontiguousarray(x[i])} for i in range(N_CORES)]
    res = run_bass_kernel_spmd(nc, in_maps, list(range(N_CORES)))
    return np.stack([res.results[i]["out"] for i in range(N_CORES)], axis=0)
